# revision 28
# baseline (speedup 1.0000x reference)
"""Trainium2 Bass kernel for nn_BinaryLinear (sign-binarized linear + BatchNorm1d,
training mode, batch statistics).

  reference:  out = BN(x @ (sign(W) * rowmask).T + bias) * gamma + beta
  shapes:     x [8192, 4096] f32, W [4096, 4096] f32, bias/gamma/beta [4096] f32

Strategy
--------
* Tensor-parallel over output features: each of the 8 cores owns 512 of the 4096
  output features.  BatchNorm reduces over the batch axis, which is entirely
  local to a core under this sharding -> no collectives.
* Each core computes out_c.T = sign(W_c.T) @ x.T as an fp16 matmul (sign
  weights are exactly +-1 in fp16; quantizing x to fp16 adds ~3e-4 rel error),
  accumulated in fp32 PSUM.  PE layout: lhsT = sign(Wt) [k=in, m=out_slice],
  rhs = xT [k=in, n=batch], producing out.T tiles [128 out, 512 batch].
* bias is dropped: BN subtracts the per-feature mean, which absorbs an additive
  per-feature bias exactly.
* sign() is computed on-device, exactly (sign(0) == 0, matching jnp.sign):
  sign(w) = clamp(w * 3e38, -1, 1).  W ships as bf16 (bf16 normals cover the
  whole fp32-normal range, so the sign is unchanged; halves the W DMA that
  competes with x at kernel start).
* The reference's pruned-row mask is a no-op: a row with sum|W| == 0 is all
  zeros, so sign(W) is already zero there (out col == 0 == mean -> BN output is
  exactly beta either way).  No mask is computed.
* BN stats stream through DVE bn_stats per PSUM tile; bn_aggr merges them.
  Final affine: scale = gamma * rsqrt(var+eps), shift = beta - mean * scale.
* The last batch chunk runs m-outer so each out-feature tile finalizes
  (bn_aggr + affine + normalize + writeout) while the remaining tiles are
  still on the PE -> the serial tail is one m-tile, not the whole output.
* Host side does only layout/dtype work: shard W, transpose, cast; upcast the
  fp16 device output to fp32.
"""

import numpy as np
import ml_dtypes

P = 128
B = 8192           # batch
IN = 4096          # in features (contraction)
OUT = 4096         # out features
NCORES = 8
OUT_S = OUT // NCORES   # 512 out features per core
KO = IN // P            # 32 contraction tiles
NCH = 512               # batch chunk = matmul free dim = one PSUM bank
NB = B // NCH           # 16 batch chunks
MT = OUT_S // P         # 4 partition tiles of out features per core
EPS = 1e-5

W_CHUNKS = [1, 3, 4, 4, 4, 4, 4, 4, 4]   # ko-tiles per W prep chunk; sums to KO
X_CHUNKS0 = [4, 4, 8, 16]       # x DMA chunking for the first batch chunk
X_CHUNKS = [16, 16]             # ... and for the rest (2 MB each, ~85% DMA eff)
NORM_CH = 2048          # normalize/write-out chunk (batch elements)

_CACHE = {}
LAST_RESULTS = None


def _build():
    import concourse.mybir as mybir
    import concourse.tile as tile
    from concourse import bacc

    f32 = mybir.dt.float32
    f16 = mybir.dt.float16
    bf16 = mybir.dt.bfloat16
    Act = mybir.ActivationFunctionType
    Alu = mybir.AluOpType

    nc = bacc.Bacc(None, target_bir_lowering=False)

    xt = nc.dram_tensor("xt", [IN, B], f16, kind="ExternalInput")
    wt = nc.dram_tensor("wt", [IN, OUT_S], bf16, kind="ExternalInput")
    gamma = nc.dram_tensor("gamma", [OUT_S], f32, kind="ExternalInput")
    beta = nc.dram_tensor("beta", [OUT_S], f32, kind="ExternalInput")
    outt = nc.dram_tensor("outt", [OUT_S, B], f16, kind="ExternalOutput")

    # i = ko*128 + p for both matmul operands; o = m*128 + p for outputs.
    xt3 = xt[:].rearrange("(ko p) b -> p ko b", p=P)
    wt3 = wt[:].rearrange("(ko p) o -> p ko o", p=P)
    outt3 = outt[:].rearrange("(m p) b -> p m b", p=P)
    gam2 = gamma[:].rearrange("(m p) -> p m", p=P)
    bet2 = beta[:].rearrange("(m p) -> p m", p=P)

    assert sum(W_CHUNKS) == KO

    with tile.TileContext(nc) as tc:
        with (
            tc.tile_pool(name="const", bufs=1) as const_pool,
            tc.tile_pool(name="ws", bufs=1) as ws_pool,
            tc.tile_pool(name="store", bufs=1) as store_pool,
            tc.tile_pool(name="wload", bufs=2) as wload_pool,
            tc.tile_pool(name="xin", bufs=4) as x_pool,
            tc.tile_pool(name="stats", bufs=1) as stats_pool,
            tc.tile_pool(name="bounce", bufs=4) as bounce_pool,
            tc.tile_pool(name="psum", bufs=8, space="PSUM") as psum_pool,
        ):
            # gamma/beta ride the SWDGE queue: they are tiny, only needed at
            # the very end, and must not delay the W/x loads on HWDGE
            gam_sb = const_pool.tile([P, MT], f32)
            bet_sb = const_pool.tile([P, MT], f32)
            nc.gpsimd.dma_start(gam_sb, gam2)
            nc.gpsimd.dma_start(bet_sb, bet2)
            eps_sb = const_pool.tile([P, 1], f32)
            nc.vector.memset(eps_sb, EPS)

            store = store_pool.tile([P, MT, B], f16)
            bnst = stats_pool.tile([P, MT, NB, 6], f32)
            mv = stats_pool.tile([P, MT, 2], f32)
            scale = stats_pool.tile([P, MT], f32)
            shift = stats_pool.tile([P, MT], f32)

            # ko -> (ws chunk tile index, index within chunk)
            ko_map = []
            w_starts = []
            k0 = 0
            for ci, nk in enumerate(W_CHUNKS):
                w_starts.append(k0)
                ko_map += [(ci, li) for li in range(nk)]
                k0 += nk
            ws_tiles = [None] * len(W_CHUNKS)

            def emit_w_chunk(ci):
                nk = W_CHUNKS[ci]
                k0 = w_starts[ci]
                wl = wload_pool.tile(
                    [P, nk, OUT_S], bf16, tag="wl", name=f"wl{ci}"
                )
                nc.sync.dma_start(wl, wt3[:, k0 : k0 + nk, :])
                # sign(w) = clamp(w * 3e38, -1, 1); exact (incl. sign(0) == 0)
                # for every normal bf16 w, and saturation still yields +-1.
                # Alternate the scale between ACT and DVE so consecutive
                # chunks' prep pipelines in parallel across both engines.
                ws_t = ws_pool.tile(
                    [P, nk, OUT_S], f16, tag=f"ws{ci}", bufs=1, name=f"ws{ci}"
                )
                if ci % 2 == 0:
                    nc.scalar.activation(wl, wl, Act.Copy, bias=0.0, scale=3e38)
                else:
                    nc.vector.tensor_scalar_mul(wl[:], wl[:], 3e38)
                nc.vector.tensor_scalar(
                    ws_t[:], wl, 1.0, -1.0, Alu.min, Alu.max
                )
                ws_tiles[ci] = ws_t

            # per-n x chunk maps: ko -> (chunk index, index within chunk)
            def x_map_for(chunks):
                mp = []
                starts = []
                k = 0
                for xi, nk in enumerate(chunks):
                    starts.append(k)
                    mp += [(xi, li) for li in range(nk)]
                    k += nk
                return mp, starts

            xmap0, xstarts0 = x_map_for(X_CHUNKS0)
            xmap, xstarts = x_map_for(X_CHUNKS)

            def emit_x_tile(n, xi):
                chunks, starts = (
                    (X_CHUNKS0, xstarts0) if n == 0 else (X_CHUNKS, xstarts)
                )
                nk = chunks[xi]
                k0 = starts[xi]
                t = x_pool.tile(
                    [P, nk, NCH], f16, tag="xck", name=f"x{n}_{xi}"
                )
                nc.sync.dma_start(
                    t,
                    xt3[:, k0 : k0 + nk, n * NCH : (n + 1) * NCH],
                )
                return t

            # Interleave W-prep chunks with the first x chunk, ordered by when
            # the PE first needs each piece (W chunk ci gates ko >=
            # w_starts[ci]; x tile xi gates ko >= xstarts0[xi]).
            xck0 = [None] * len(X_CHUNKS0)
            emit_w_chunk(0)
            xck0[0] = emit_x_tile(0, 0)
            emit_w_chunk(1)
            emit_w_chunk(2)
            xck0[1] = emit_x_tile(0, 1)
            emit_w_chunk(3)
            xck0[2] = emit_x_tile(0, 2)
            emit_w_chunk(4)
            emit_w_chunk(5)
            emit_w_chunk(6)
            xck0[3] = emit_x_tile(0, 3)
            emit_w_chunk(7)
            emit_w_chunk(8)

            def drain_psum(m, n, ps_m):
                bsl = slice(n * NCH, (n + 1) * NCH)
                nc.scalar.activation(store[:, m, bsl], ps_m, Act.Copy)
                nc.vector.bn_stats(bnst[:, m, n, :], ps_m)

            def finalize_m(m, act_chunks=()):
                """bn_aggr + affine coefficients + normalize + write out."""
                sm = slice(m, m + 1)
                nc.vector.bn_aggr(mv[:, m, :], bnst[:, m, :, :])
                # rstd = 1 / sqrt(var + eps)
                nc.scalar.activation(
                    scale[:, sm], mv[:, m, 1:2], Act.Sqrt,
                    bias=eps_sb[:], scale=1.0,
                )
                nc.vector.reciprocal(scale[:, sm], scale[:, sm])
                nc.vector.tensor_tensor(
                    scale[:, sm], scale[:, sm], gam_sb[:, sm], Alu.mult
                )
                # shift = beta - mean * scale
                nc.vector.tensor_tensor(
                    shift[:, sm], mv[:, m, 0:1], scale[:, sm], Alu.mult
                )
                nc.vector.tensor_tensor(
                    shift[:, sm], bet_sb[:, sm], shift[:, sm], Alu.subtract
                )
                # DVE normalize (fp16 2x mode beats the ACT LUT path and keeps
                # ACT free for the PSUM drains); near the kernel tail ACT is
                # idle, so selected chunks go there to unload DVE.
                for ic, c0 in enumerate(range(0, B, NORM_CH)):
                    bb = bounce_pool.tile([P, NORM_CH], f16, tag="bb")
                    src = store[:, m, c0 : c0 + NORM_CH]
                    if ic in act_chunks:
                        nc.scalar.activation(
                            bb, src, Act.Identity,
                            bias=shift[:, sm], scale=scale[:, sm],
                        )
                    else:
                        nc.vector.tensor_scalar(
                            bb, src, scale[:, sm], shift[:, sm],
                            Alu.mult, Alu.add,
                        )
                    nc.sync.dma_start(outt3[:, m, c0 : c0 + NORM_CH], bb)

            # ---- main loop: out.T accumulation + streaming BN stats ----
            for n in range(NB):
                if n == 0:
                    xck, xm = xck0, xmap0
                else:
                    xck = [emit_x_tile(n, xi) for xi in range(len(X_CHUNKS))]
                    xm = xmap

                if n < NB - 1:
                    # ko outer / m inner: x tiles are released early (prefetch
                    # window) and the PE never waits on DMA mid-chunk
                    ps = [
                        psum_pool.tile([P, NCH], f32, tag="ps", name=f"ps{n}_{m}")
                        for m in range(MT)
                    ]
                    for ko in range(KO):
                        ci, li = ko_map[ko]
                        xi, xl = xm[ko]
                        for m in range(MT):
                            nc.tensor.matmul(
                                ps[m],
                                lhsT=ws_tiles[ci][:, li, m * P : (m + 1) * P],
                                rhs=xck[xi][:, xl, :],
                                start=(ko == 0),
                                stop=(ko == KO - 1),
                            )
                    for m in range(MT):
                        drain_psum(m, n, ps[m])
                else:
                    # last chunk: m outer, so each m-tile finalizes (stats,
                    # affine, normalize, DMA out) while later m-tiles are
                    # still on the PE -> the serial tail is one m-tile
                    for m in range(MT):
                        ps_m = psum_pool.tile(
                            [P, NCH], f32, tag="ps", name=f"ps{n}_{m}"
                        )
                        for ko in range(KO):
                            ci, li = ko_map[ko]
                            xi, xl = xm[ko]
                            nc.tensor.matmul(
                                ps_m,
                                lhsT=ws_tiles[ci][:, li, m * P : (m + 1) * P],
                                rhs=xck[xi][:, xl, :],
                                start=(ko == 0),
                                stop=(ko == KO - 1),
                            )
                        drain_psum(m, n, ps_m)
                        # m2's last chunk and m3's first go to ACT so DVE is
                        # clear for m3's critical stats->coeffs->normalize chain
                        finalize_m(
                            m,
                            act_chunks=(
                                (3,) if m == MT - 2 else (0,) if m == MT - 1 else ()
                            ),
                        )

    nc.compile()
    return nc


def _get_nc():
    if "nc" not in _CACHE:
        _CACHE["nc"] = _build()
    return _CACHE["nc"]


def kernel(x, weight, bias, gamma, beta):
    global LAST_RESULTS
    from concourse.bass_utils import run_bass_kernel_spmd

    x = np.asarray(x, dtype=np.float32)
    weight = np.asarray(weight, dtype=np.float32)
    gamma = np.asarray(gamma, dtype=np.float32)
    beta = np.asarray(beta, dtype=np.float32)
    # bias is mathematically absorbed by the BN mean subtraction -> unused

    nc = _get_nc()

    # host-side layout/dtype prep only
    xt = np.ascontiguousarray(x.astype(np.float16).T)  # [IN, B] fp16
    wbt = np.ascontiguousarray(weight.T.astype(ml_dtypes.bfloat16))  # [IN, OUT]
    in_maps = []
    for c in range(NCORES):
        osl = slice(OUT_S * c, OUT_S * (c + 1))
        in_maps.append(
            {
                "xt": xt,
                "wt": np.ascontiguousarray(wbt[:, osl]),  # [IN, OUT_S] bf16
                "gamma": np.ascontiguousarray(gamma[osl]),
                "beta": np.ascontiguousarray(beta[osl]),
            }
        )

    res = run_bass_kernel_spmd(nc, in_maps, core_ids=list(range(NCORES)))
    LAST_RESULTS = res

    out = np.empty((B, OUT), dtype=np.float32)
    for c in range(NCORES):
        out[:, OUT_S * c : OUT_S * (c + 1)] = (
            res.results[c]["outt"].astype(np.float32).T
        )
    return out


# revision 33
# speedup vs baseline: 1.0098x; 1.0098x over previous
"""Trainium2 Bass kernel for nn_BinaryLinear (sign-binarized linear + BatchNorm1d,
training mode, batch statistics).

  reference:  out = BN(x @ (sign(W) * rowmask).T + bias) * gamma + beta
  shapes:     x [8192, 4096] f32, W [4096, 4096] f32, bias/gamma/beta [4096] f32

Strategy
--------
* Tensor-parallel over output features: each of the 8 cores owns 512 of the 4096
  output features.  BatchNorm reduces over the batch axis, which is entirely
  local to a core under this sharding -> no collectives.
* Each core computes out_c.T = sign(W_c.T) @ x.T as an fp16 matmul (sign
  weights are exactly +-1 in fp16; quantizing x to fp16 adds ~3e-4 rel error),
  accumulated in fp32 PSUM.  PE layout: lhsT = sign(Wt) [k=in, m=out_slice],
  rhs = xT [k=in, n=batch], producing out.T tiles [128 out, 512 batch].
* bias is dropped: BN subtracts the per-feature mean, which absorbs an additive
  per-feature bias exactly.
* sign() is computed on-device, exactly (sign(0) == 0, matching jnp.sign):
  sign(w) = clamp(w * 3e38, -1, 1).  W ships as bf16 (bf16 normals cover the
  whole fp32-normal range, so the sign is unchanged; halves the W DMA that
  competes with x at kernel start).
* The reference's pruned-row mask is a no-op: a row with sum|W| == 0 is all
  zeros, so sign(W) is already zero there (out col == 0 == mean -> BN output is
  exactly beta either way).  No mask is computed.
* BN stats stream through DVE bn_stats per PSUM tile; bn_aggr merges them.
  Final affine: scale = gamma * rsqrt(var+eps), shift = beta - mean * scale.
* The last batch chunk runs m-outer so each out-feature tile finalizes
  (bn_aggr + affine + normalize + writeout) while the remaining tiles are
  still on the PE -> the serial tail is one m-tile, not the whole output.
* Host side does only layout/dtype work: shard W, transpose, cast; upcast the
  fp16 device output to fp32.
"""

import sys
import types

import numpy as np
import ml_dtypes

P = 128
B = 8192           # batch
IN = 4096          # in features (contraction)
OUT = 4096         # out features
NCORES = 8
OUT_S = OUT // NCORES   # 512 out features per core
KO = IN // P            # 32 contraction tiles
NCH = 512               # batch chunk = matmul free dim = one PSUM bank
NB = B // NCH           # 16 batch chunks
MT = OUT_S // P         # 4 partition tiles of out features per core
EPS = 1e-5

W_CHUNKS = [1, 3, 4, 4, 4, 4, 4, 4, 4]   # ko-tiles per W prep chunk; sums to KO
X_CHUNKS0 = [4, 4, 8, 8, 8]     # x DMA chunking for the first batch chunk
X_CHUNKS = [8, 8, 8, 8]         # ... and for the rest
NORM_CH = 2048          # normalize/write-out chunk (batch elements)

_CACHE = {}
LAST_RESULTS = None


def _build():
    import concourse.mybir as mybir
    import concourse.tile as tile
    from concourse import bacc

    f32 = mybir.dt.float32
    f16 = mybir.dt.float16
    bf16 = mybir.dt.bfloat16
    Act = mybir.ActivationFunctionType
    Alu = mybir.AluOpType

    nc = bacc.Bacc(None, target_bir_lowering=False)

    xt = nc.dram_tensor("xt", [IN, B], f16, kind="ExternalInput")
    wt = nc.dram_tensor("wt", [IN, OUT_S], bf16, kind="ExternalInput")
    gamma = nc.dram_tensor("gamma", [OUT_S], f32, kind="ExternalInput")
    beta = nc.dram_tensor("beta", [OUT_S], f32, kind="ExternalInput")
    outt = nc.dram_tensor("outt", [OUT_S, B], f16, kind="ExternalOutput")

    # i = ko*128 + p for both matmul operands; o = m*128 + p for outputs.
    xt3 = xt[:].rearrange("(ko p) b -> p ko b", p=P)
    wt3 = wt[:].rearrange("(ko p) o -> p ko o", p=P)
    outt3 = outt[:].rearrange("(m p) b -> p m b", p=P)
    gam2 = gamma[:].rearrange("(m p) -> p m", p=P)
    bet2 = beta[:].rearrange("(m p) -> p m", p=P)

    assert sum(W_CHUNKS) == KO

    with tile.TileContext(nc) as tc:
        with (
            tc.tile_pool(name="const", bufs=1) as const_pool,
            tc.tile_pool(name="ws", bufs=1) as ws_pool,
            tc.tile_pool(name="store", bufs=1) as store_pool,
            tc.tile_pool(name="wload", bufs=2) as wload_pool,
            tc.tile_pool(name="xin", bufs=7) as x_pool,
            tc.tile_pool(name="stats", bufs=1) as stats_pool,
            tc.tile_pool(name="bounce", bufs=4) as bounce_pool,
            tc.tile_pool(name="psum", bufs=8, space="PSUM") as psum_pool,
        ):
            # gamma/beta ride the SWDGE queue: they are tiny, only needed at
            # the very end, and must not delay the W/x loads on HWDGE
            gam_sb = const_pool.tile([P, MT], f32)
            bet_sb = const_pool.tile([P, MT], f32)
            nc.gpsimd.dma_start(gam_sb, gam2)
            nc.gpsimd.dma_start(bet_sb, bet2)
            eps_sb = const_pool.tile([P, 1], f32)
            nc.vector.memset(eps_sb, EPS)

            store = store_pool.tile([P, MT, B], f16)
            bnst = stats_pool.tile([P, MT, NB, 6], f32)
            mv = stats_pool.tile([P, MT, 2], f32)
            scale = stats_pool.tile([P, MT], f32)
            shift = stats_pool.tile([P, MT], f32)

            # ko -> (ws chunk tile index, index within chunk)
            ko_map = []
            w_starts = []
            k0 = 0
            for ci, nk in enumerate(W_CHUNKS):
                w_starts.append(k0)
                ko_map += [(ci, li) for li in range(nk)]
                k0 += nk
            ws_tiles = [None] * len(W_CHUNKS)

            def emit_w_chunk(ci):
                nk = W_CHUNKS[ci]
                k0 = w_starts[ci]
                wl = wload_pool.tile(
                    [P, nk, OUT_S], bf16, tag="wl", name=f"wl{ci}"
                )
                nc.sync.dma_start(wl, wt3[:, k0 : k0 + nk, :])
                # sign(w) = clamp(w * 3e38, -1, 1); exact (incl. sign(0) == 0)
                # for every normal bf16 w, and saturation still yields +-1.
                # Alternate the scale between ACT and DVE so consecutive
                # chunks' prep pipelines in parallel across both engines.
                ws_t = ws_pool.tile(
                    [P, nk, OUT_S], f16, tag=f"ws{ci}", bufs=1, name=f"ws{ci}"
                )
                if ci % 2 == 0:
                    nc.scalar.activation(wl, wl, Act.Copy, bias=0.0, scale=3e38)
                else:
                    nc.vector.tensor_scalar_mul(wl[:], wl[:], 3e38)
                nc.vector.tensor_scalar(
                    ws_t[:], wl, 1.0, -1.0, Alu.min, Alu.max
                )
                ws_tiles[ci] = ws_t

            # per-n x chunk maps: ko -> (chunk index, index within chunk)
            def x_map_for(chunks):
                mp = []
                starts = []
                k = 0
                for xi, nk in enumerate(chunks):
                    starts.append(k)
                    mp += [(xi, li) for li in range(nk)]
                    k += nk
                return mp, starts

            xmap0, xstarts0 = x_map_for(X_CHUNKS0)
            xmap, xstarts = x_map_for(X_CHUNKS)

            def emit_x_tile(n, xi):
                chunks, starts = (
                    (X_CHUNKS0, xstarts0) if n == 0 else (X_CHUNKS, xstarts)
                )
                nk = chunks[xi]
                k0 = starts[xi]
                t = x_pool.tile(
                    [P, nk, NCH], f16, tag="xck", name=f"x{n}_{xi}"
                )
                nc.sync.dma_start(
                    t,
                    xt3[:, k0 : k0 + nk, n * NCH : (n + 1) * NCH],
                )
                return t

            # Interleave W-prep chunks with the first x chunk, ordered by when
            # the PE first needs each piece (W chunk ci gates ko >=
            # w_starts[ci]; x tile xi gates ko >= xstarts0[xi]).
            xck0 = [None] * len(X_CHUNKS0)
            emit_w_chunk(0)
            xck0[0] = emit_x_tile(0, 0)
            emit_w_chunk(1)
            emit_w_chunk(2)
            xck0[1] = emit_x_tile(0, 1)
            emit_w_chunk(3)
            xck0[2] = emit_x_tile(0, 2)
            emit_w_chunk(4)
            emit_w_chunk(5)
            xck0[3] = emit_x_tile(0, 3)
            emit_w_chunk(6)
            emit_w_chunk(7)
            xck0[4] = emit_x_tile(0, 4)
            emit_w_chunk(8)

            def drain_psum(m, n, ps_m):
                bsl = slice(n * NCH, (n + 1) * NCH)
                nc.scalar.activation(store[:, m, bsl], ps_m, Act.Copy)
                nc.vector.bn_stats(bnst[:, m, n, :], ps_m)

            def finalize_m(m, act_chunks=()):
                """bn_aggr + affine coefficients + normalize + write out."""
                sm = slice(m, m + 1)
                nc.vector.bn_aggr(mv[:, m, :], bnst[:, m, :, :])
                # rstd = 1 / sqrt(var + eps)
                nc.scalar.activation(
                    scale[:, sm], mv[:, m, 1:2], Act.Sqrt,
                    bias=eps_sb[:], scale=1.0,
                )
                nc.vector.reciprocal(scale[:, sm], scale[:, sm])
                nc.vector.tensor_tensor(
                    scale[:, sm], scale[:, sm], gam_sb[:, sm], Alu.mult
                )
                # shift = beta - mean * scale
                nc.vector.tensor_tensor(
                    shift[:, sm], mv[:, m, 0:1], scale[:, sm], Alu.mult
                )
                nc.vector.tensor_tensor(
                    shift[:, sm], bet_sb[:, sm], shift[:, sm], Alu.subtract
                )
                # DVE normalize (fp16 2x mode beats the ACT LUT path and keeps
                # ACT free for the PSUM drains); near the kernel tail ACT is
                # idle, so selected chunks go there to unload DVE.
                for ic, c0 in enumerate(range(0, B, NORM_CH)):
                    bb = bounce_pool.tile([P, NORM_CH], f16, tag="bb")
                    src = store[:, m, c0 : c0 + NORM_CH]
                    if ic in act_chunks:
                        nc.scalar.activation(
                            bb, src, Act.Identity,
                            bias=shift[:, sm], scale=scale[:, sm],
                        )
                    else:
                        nc.vector.tensor_scalar(
                            bb, src, scale[:, sm], shift[:, sm],
                            Alu.mult, Alu.add,
                        )
                    nc.sync.dma_start(outt3[:, m, c0 : c0 + NORM_CH], bb)

            # ---- main loop: out.T accumulation + streaming BN stats ----
            for n in range(NB):
                if n == 0:
                    xck, xm = xck0, xmap0
                else:
                    xck = [emit_x_tile(n, xi) for xi in range(len(X_CHUNKS))]
                    xm = xmap

                if n < NB - 1:
                    # ko outer / m inner: x tiles are released early (prefetch
                    # window) and the PE never waits on DMA mid-chunk
                    ps = [
                        psum_pool.tile([P, NCH], f32, tag="ps", name=f"ps{n}_{m}")
                        for m in range(MT)
                    ]
                    for ko in range(KO):
                        ci, li = ko_map[ko]
                        xi, xl = xm[ko]
                        for m in range(MT):
                            nc.tensor.matmul(
                                ps[m],
                                lhsT=ws_tiles[ci][:, li, m * P : (m + 1) * P],
                                rhs=xck[xi][:, xl, :],
                                start=(ko == 0),
                                stop=(ko == KO - 1),
                            )
                    for m in range(MT):
                        drain_psum(m, n, ps[m])
                else:
                    # last chunk: m outer, so each m-tile finalizes (stats,
                    # affine, normalize, DMA out) while later m-tiles are
                    # still on the PE -> the serial tail is one m-tile
                    for m in range(MT):
                        ps_m = psum_pool.tile(
                            [P, NCH], f32, tag="ps", name=f"ps{n}_{m}"
                        )
                        for ko in range(KO):
                            ci, li = ko_map[ko]
                            xi, xl = xm[ko]
                            nc.tensor.matmul(
                                ps_m,
                                lhsT=ws_tiles[ci][:, li, m * P : (m + 1) * P],
                                rhs=xck[xi][:, xl, :],
                                start=(ko == 0),
                                stop=(ko == KO - 1),
                            )
                        drain_psum(m, n, ps_m)
                        # m2's last chunk and m3's first go to ACT so DVE is
                        # clear for m3's critical stats->coeffs->normalize chain
                        finalize_m(
                            m,
                            act_chunks=(
                                (3,) if m == MT - 2 else (0,) if m == MT - 1 else ()
                            ),
                        )

    nc.compile()
    return nc


def _get_nc():
    if "nc" not in _CACHE:
        _CACHE["nc"] = _build()
    return _CACHE["nc"]


def _ensure_axon_hooks():
    """Some containers lack antenv.axon_hooks; run_bass_kernel_spmd imports it
    when tracing is requested (e.g. BASS_TRACE=1).  Provide it, and register
    the ctypes NTFF hook when the boot shim is available, so tracing either
    works or degrades gracefully instead of raising ImportError."""
    try:
        import antenv.axon_hooks  # noqa: F401
        return
    except ImportError:
        pass
    mod = types.ModuleType("antenv.axon_hooks")
    mod._hook = None
    mod.set_axon_ntff_profile_hook = lambda h: setattr(mod, "_hook", h)
    mod.get_axon_ntff_profile_hook = lambda: mod._hook
    sys.modules["antenv.axon_hooks"] = mod
    try:
        import antenv

        antenv.axon_hooks = mod
    except ImportError:
        pass
    try:
        from trn_agent_boot.trn_boot import _ntff_profile_via_ctypes

        mod._hook = _ntff_profile_via_ctypes("/opt/axon/libaxon_pjrt.so")
    except Exception:
        pass


def kernel(x, weight, bias, gamma, beta):
    global LAST_RESULTS
    _ensure_axon_hooks()
    from concourse.bass_utils import run_bass_kernel_spmd

    x = np.asarray(x, dtype=np.float32)
    weight = np.asarray(weight, dtype=np.float32)
    gamma = np.asarray(gamma, dtype=np.float32)
    beta = np.asarray(beta, dtype=np.float32)
    # bias is mathematically absorbed by the BN mean subtraction -> unused

    nc = _get_nc()

    # host-side layout/dtype prep only
    xt = np.ascontiguousarray(x.astype(np.float16).T)  # [IN, B] fp16
    wbt = np.ascontiguousarray(weight.T.astype(ml_dtypes.bfloat16))  # [IN, OUT]
    in_maps = []
    for c in range(NCORES):
        osl = slice(OUT_S * c, OUT_S * (c + 1))
        in_maps.append(
            {
                "xt": xt,
                "wt": np.ascontiguousarray(wbt[:, osl]),  # [IN, OUT_S] bf16
                "gamma": np.ascontiguousarray(gamma[osl]),
                "beta": np.ascontiguousarray(beta[osl]),
            }
        )

    res = run_bass_kernel_spmd(nc, in_maps, core_ids=list(range(NCORES)))
    LAST_RESULTS = res

    out = np.empty((B, OUT), dtype=np.float32)
    for c in range(NCORES):
        out[:, OUT_S * c : OUT_S * (c + 1)] = (
            res.results[c]["outt"].astype(np.float32).T
        )
    return out
